# revision 43
# baseline (speedup 1.0000x reference)
"""Trainium2 Bass kernel for DynGraphBlock (gnn_message_passing).

Per sample n (x: [C=128, T=2048], A_prev: [C, C], gamma scalar):
  1. corr affinity  A0 = relu(corr(x))           (Pearson corr over T, ddof=1)
  2. top-8 sparsify per row
  3. normalize: sym + self-loop + row-normalize
  4. EMA with A_prev -> A
  5. x_out = x + gamma * (A @ x)
Returns (x_out, A).

Mapping:
  - Pure data parallel: 256 samples -> 8 NeuronCores x 32 samples.
  - Gram G = x @ x.T via PE with x transposed on PE (16 [128,128] transposes),
    augmented with a ones column so the same matmuls also produce row sums S.
    corr = (G - u u^T) * w_c * w_k / T with u = S/sqrt(T), w = 1/(std+eps);
    u/w rows come from a PE transpose + gpsimd partition_broadcast, and the
    rank-1 mean correction and scaling run on DVE.
  - top-8 threshold via the DVE Max (top-8 per partition) instruction: keep
    entries >= 8th largest (ties only occur among zero/negative entries,
    where the masked product is 0 either way, matching jax top_k).
  - x_out = x + gamma*A@x is a single PE matmul with W = gamma*A^T + I; it
    runs in fp32r (tf32) on a rounded copy of x (A/corr keep full fp32).
  - Tile priority nudges: per-sample stats promoted, post-Gram tail demoted,
    so PE stays busy with the next sample's transposes/Gram during the
    current sample's DVE chain.
"""

import numpy as np
import sys

if "/opt/trn_rl_repo" not in sys.path:
    sys.path.insert(0, "/opt/trn_rl_repo")

N, C, T = 256, 128, 2048
NCORES = 8
NS = N // NCORES            # samples per core
TOPK = 8
EMA_ALPHA = 0.8
EPS = 1e-6
TB = T // C                 # 16 t-blocks of 128
ZCHUNK = 512                # moving free dim per x_out matmul
SQT = float(np.sqrt(np.float64(T)))

_CACHE = {}


def _build(ns, z_f32r=False, stage=99, reps=1):
    import concourse.bacc as bacc
    import concourse.mybir as mybir
    import concourse.tile as tile

    f32 = mybir.dt.float32
    f32r = mybir.dt.float32r
    op = mybir.AluOpType

    nc = bacc.Bacc("TRN2", target_bir_lowering=False, debug=False,
                   enable_asserts=False, num_devices=NCORES)
    x_d = nc.dram_tensor("x", [ns, C, T], f32, kind="ExternalInput")
    ap_d = nc.dram_tensor("a_prev", [ns, C, C], f32, kind="ExternalInput")
    id_d = nc.dram_tensor("ident", [C, C], f32, kind="ExternalInput")
    gv_d = nc.dram_tensor("gvec", [C, 1], f32, kind="ExternalInput")
    xo_d = nc.dram_tensor("x_out", [ns, C, T], f32, kind="ExternalOutput")
    ao_d = nc.dram_tensor("a_out", [ns, C, C], f32, kind="ExternalOutput")

    with tile.TileContext(nc) as tc:
        with (
            tc.tile_pool(name="const", bufs=1) as constp,
            tc.tile_pool(name="xin", bufs=4) as xinp,
            tc.tile_pool(name="xt", bufs=4) as xtp,
            tc.tile_pool(name="apin", bufs=3) as apinp,
            tc.tile_pool(name="xout", bufs=3) as xoutp,
            tc.tile_pool(name="wrk", bufs=4) as wrk,
            tc.tile_pool(name="tp_ps", bufs=3, space="PSUM") as tpps,
            tc.tile_pool(name="g_ps", bufs=2, space="PSUM") as gps,
            tc.tile_pool(name="st_ps", bufs=1, space="PSUM") as stps,
            tc.tile_pool(name="z_ps", bufs=2, space="PSUM") as zps,
        ):
            ident = constp.tile([C, C], f32, tag="ident")
            nc.sync.dma_start(ident[:], id_d.ap())
            gvec = constp.tile([C, 1], f32, tag="gvec")
            nc.sync.dma_start(gvec[:], gv_d.ap())

            for _it in range(reps * ns):
                s = _it % ns
                # ---- loads ----
                x_sb = xinp.tile([C, T], f32, tag="x")
                nc.sync.dma_start(x_sb[:], x_d.ap()[s])
                if z_f32r:
                    # tf32-rounded copy of x for the fast x_out matmul; the
                    # Gram/corr path keeps full fp32 x. gpsimd is idle.
                    x_r = xinp.tile([C, T], f32r, tag="xr")
                    nc.gpsimd.tensor_copy(x_r[:], x_sb[:])
                ap_sb = apinp.tile([C, C], f32, tag="ap")
                nc.sync.dma_start(ap_sb[:], ap_d.ap()[s])
                ap8 = wrk.tile([C, C], f32, tag="ap8")
                nc.vector.tensor_scalar(ap8[:], ap_sb[:], EMA_ALPHA, None,
                                        op.mult)

                # ---- transpose x into xT blocks (+ ones columns) ----
                # xT layout: 16 blocks of 129 cols: [128 cols x^T block | 1.0]
                xT = xtp.tile([C, TB * (C + 1)], f32, tag="xT")
                xT3 = xT.rearrange("p (b c) -> p b c", c=C + 1)
                nc.gpsimd.memset(xT3[:, :, C:C + 1], 1.0)
                for g in range(TB // 4):
                    tp = tpps.tile([C, 4 * C], f32, tag="tp")
                    for j in range(4):
                        blk = 4 * g + j
                        nc.tensor.transpose(
                            tp[:, j * C:(j + 1) * C],
                            x_sb[:, blk * C:(blk + 1) * C], ident[:])
                    src = tp.rearrange("p (b c) -> p b c", c=C)
                    dst = xT3[:, 4 * g:4 * (g + 1), 0:C]
                    if g % 2 == 0:
                        nc.vector.tensor_copy(dst, src)
                    else:
                        nc.scalar.copy(dst, src)

                # ---- Gram (+ row sums in col 128) ----
                G = gps.tile([C, C + 1], f32, tag="g")
                for k in range(TB):
                    nc.tensor.matmul(
                        G[:], xT[:, k * (C + 1):k * (C + 1) + C],
                        xT[:, k * (C + 1):k * (C + 1) + C + 1],
                        start=(k == 0), stop=(k == TB - 1),
                        skip_group_check=True)

                if stage <= 1:
                    dmp = wrk.tile([C, C], f32, tag="dmp")
                    nc.vector.tensor_copy(dmp[:], G[:, 0:C])
                    nc.sync.dma_start(ao_d.ap()[s], dmp[:])
                    xo_sb = xoutp.tile([C, T], f32, tag="xo")
                    nc.vector.tensor_copy(xo_sb[:], x_sb[:])
                    nc.sync.dma_start(xo_d.ap()[s], xo_sb[:])
                    continue

                # ---- stats: u = S/sqrt(T), w = 1/(std+eps) ----
                _sctx = tc.high_priority(offset=35)
                _sctx.__enter__()
                scr = wrk.tile([C, C], f32, tag="scr")
                Q = wrk.tile([C, 1], f32, tag="q")
                nc.vector.tensor_tensor(scr[:], G[:, 0:C], ident[:], op.mult)
                nc.vector.reduce_sum(Q[:], scr[:], axis=mybir.AxisListType.X)
                uw = wrk.tile([C, 3], f32, tag="uw")
                # col1 = u = S/sqrt(T); col0 = -u; col2 = w
                nc.vector.tensor_scalar(uw[:, 1:2], G[:, C:C + 1], 1.0 / SQT,
                                        None, op.mult)
                sq = wrk.tile([C, 1], f32, tag="sq")
                nc.vector.tensor_tensor(sq[:], uw[:, 1:2], uw[:, 1:2], op.mult)
                varnum = wrk.tile([C, 1], f32, tag="varnum")
                nc.vector.tensor_tensor(varnum[:], Q[:], sq[:], op.subtract)
                stdv = wrk.tile([C, 1], f32, tag="stdv")
                nc.scalar.activation(stdv[:], varnum[:],
                                     mybir.ActivationFunctionType.Sqrt,
                                     bias=0.0, scale=1.0 / (T - 1))
                stde = wrk.tile([C, 1], f32, tag="stde")
                nc.vector.tensor_scalar(stde[:], stdv[:], EPS, None, op.add)
                nc.vector.reciprocal(uw[:, 2:3], stde[:])
                _sctx.__exit__(None, None, None)
                _pctx = tc.high_priority(offset=-30)
                _pctx.__enter__()

                # ---- rows: transpose u, w columns to [1, 128] rows ----
                R = stps.tile([1, 2 * C], f32, tag="st")
                for jj, j in enumerate((1, 2)):
                    nc.tensor.transpose(R[0:1, jj * C:(jj + 1) * C],
                                        uw[:, j:j + 1], ident[:])
                uwrow = wrk.tile([1, 2 * C], f32, tag="uwrow")
                nc.scalar.copy(uwrow[:], R[:])

                if stage <= 2:
                    dmp = wrk.tile([C, C], f32, tag="dmp")
                    nc.vector.tensor_copy(dmp[:], G[:, 0:C])
                    nc.sync.dma_start(ao_d.ap()[s], dmp[:])
                    xo_sb = xoutp.tile([C, T], f32, tag="xo")
                    nc.vector.tensor_copy(xo_sb[:], x_sb[:])
                    nc.sync.dma_start(xo_d.ap()[s], xo_sb[:])
                    continue

                # Urep[c, k] = u_k ; Wrep[c, k] = w_k
                urep = wrk.tile([C, C], f32, tag="urep")
                nc.gpsimd.partition_broadcast(urep[:], uwrow[0:1, 0:C])
                wrep = wrk.tile([C, C], f32, tag="wrep")
                nc.gpsimd.partition_broadcast(wrep[:], uwrow[0:1, C:2 * C])

                # ---- Gc = G - u u^T ; t2 = Gc * w_k ; top-8 threshold ----
                uu = wrk.tile([C, C], f32, tag="uu")
                nc.vector.tensor_scalar(uu[:], urep[:], uw[:, 1:2], None,
                                        op.mult)
                gc = wrk.tile([C, C], f32, tag="gc")
                nc.vector.tensor_tensor(gc[:], G[:, 0:C], uu[:], op.subtract)
                t2 = wrk.tile([C, C], f32, tag="t2")
                nc.vector.tensor_tensor(t2[:], gc[:], wrep[:], op.mult)
                top8 = wrk.tile([C, 8], f32, tag="top8")
                nc.vector.max(top8[:], t2[:])
                mask = wrk.tile([C, C], f32, tag="mask")
                nc.vector.tensor_scalar(mask[:], t2[:], top8[:, 7:8], None,
                                        op.is_ge)
                m1 = wrk.tile([C, C], f32, tag="m1")
                nc.vector.tensor_tensor(m1[:], t2[:], mask[:], op.mult)
                # Asp = relu(m1 * (w_c / T))
                w0T = wrk.tile([C, 1], f32, tag="w0T")
                nc.vector.tensor_scalar(w0T[:], uw[:, 2:3], 1.0 / T, None,
                                        op.mult)
                Asp = wrk.tile([C, C], f32, tag="asp")
                nc.vector.tensor_scalar(Asp[:], m1[:], w0T[:], 0.0,
                                        op.mult, op.max)

                if stage <= 3:
                    nc.sync.dma_start(ao_d.ap()[s], Asp[:])
                    xo_sb = xoutp.tile([C, T], f32, tag="xo")
                    nc.vector.tensor_copy(xo_sb[:], x_sb[:])
                    nc.sync.dma_start(xo_d.ap()[s], xo_sb[:])
                    continue

                # ---- symmetrize + degree ----
                AspT = stps.tile([C, C], f32, tag="st")
                nc.tensor.transpose(AspT[:], Asp[:], ident[:])
                S2 = wrk.tile([C, C], f32, tag="s2")
                nc.vector.tensor_tensor(S2[:], Asp[:], AspT[:], op.add)
                M05 = wrk.tile([C, C], f32, tag="m05")
                nc.vector.tensor_scalar(M05[:], S2[:], 0.5, None, op.mult)
                deg0 = wrk.tile([C, 1], f32, tag="deg0")
                nc.vector.reduce_sum(deg0[:], S2[:], axis=mybir.AxisListType.X)
                # deg = 0.5 * rowsum(S2) + 1 (self-loop) + EPS
                degi = wrk.tile([C, 1], f32, tag="degi")
                nc.vector.tensor_scalar(degi[:], deg0[:], 0.5, 1.0 + EPS,
                                        op.mult, op.add)
                dinv = wrk.tile([C, 1], f32, tag="dinv")
                nc.vector.reciprocal(dinv[:], degi[:])
                d02 = wrk.tile([C, 1], f32, tag="d02")
                nc.vector.tensor_scalar(d02[:], dinv[:], 1.0 - EMA_ALPHA, None,
                                        op.mult)

                # ---- EMA: Afin = alpha*A_prev + (M05 + I) * d02_c ----
                f1 = wrk.tile([C, C], f32, tag="f1")
                nc.vector.tensor_scalar(f1[:], M05[:], d02[:], None, op.mult)
                f2 = wrk.tile([C, C], f32, tag="f2")
                nc.vector.tensor_scalar(f2[:], ident[:], d02[:], None, op.mult)
                f3 = wrk.tile([C, C], f32, tag="f3")
                nc.vector.tensor_tensor(f3[:], f1[:], f2[:], op.add)
                Afin = wrk.tile([C, C], f32, tag="afin")
                nc.vector.tensor_tensor(Afin[:], f3[:], ap8[:], op.add)
                nc.sync.dma_start(ao_d.ap()[s], Afin[:])

                if stage <= 4:
                    xo_sb = xoutp.tile([C, T], f32, tag="xo")
                    nc.vector.tensor_copy(xo_sb[:], x_sb[:])
                    nc.sync.dma_start(xo_d.ap()[s], xo_sb[:])
                    continue

                # ---- W = gamma * Afin^T + I ; x_out = W^T @ x ----
                AfinT = stps.tile([C, C], f32, tag="st")
                nc.tensor.transpose(AfinT[:], Afin[:], ident[:])
                Wt = wrk.tile([C, C], f32, tag="wt")
                nc.vector.tensor_scalar(Wt[:], AfinT[:], gvec[:], None,
                                        op.mult)
                W = wrk.tile([C, C], f32r if z_f32r else f32, tag="w")
                nc.vector.tensor_tensor(W[:], Wt[:], ident[:], op.add)

                xo_sb = xoutp.tile([C, T], f32, tag="xo")
                xz = x_r if z_f32r else x_sb
                for j in range(T // ZCHUNK):
                    zp = zps.tile([C, ZCHUNK], f32, tag="z")
                    nc.tensor.matmul(
                        zp[:], W[:],
                        xz[:, j * ZCHUNK:(j + 1) * ZCHUNK],
                        start=True, stop=True)
                    nc.scalar.copy(xo_sb[:, j * ZCHUNK:(j + 1) * ZCHUNK],
                                   zp[:])
                nc.sync.dma_start(xo_d.ap()[s], xo_sb[:])
                _pctx.__exit__(None, None, None)

    nc.compile()
    return nc


def _get_nc(ns=NS, z_f32r=False, stage=99, reps=1):
    key = (ns, z_f32r, stage, reps)
    if key not in _CACHE:
        _CACHE[key] = _build(ns, z_f32r, stage, reps)
    return _CACHE[key]


def _run(x, A_prev, gamma, ns, ncores, z_f32r=False, trace=False, stage=99):
    from concourse.bass_utils import run_bass_kernel_spmd

    nc = _get_nc(ns, z_f32r, stage)
    x = np.ascontiguousarray(x, dtype=np.float32)
    A_prev = np.ascontiguousarray(A_prev, dtype=np.float32)
    ident = np.eye(C, dtype=np.float32)
    gvec = np.full((C, 1), np.float32(gamma), dtype=np.float32)
    in_maps = []
    for c in range(ncores):
        sl = slice(c * ns, (c + 1) * ns)
        in_maps.append({"x": x[sl], "a_prev": A_prev[sl],
                        "ident": ident, "gvec": gvec})
    res = run_bass_kernel_spmd(nc, in_maps, core_ids=list(range(ncores)),
                               trace=trace)
    x_out = np.concatenate([res.results[c]["x_out"] for c in range(ncores)], 0)
    a_out = np.concatenate([res.results[c]["a_out"] for c in range(ncores)], 0)
    return (x_out, a_out), res


_RUNNERS = {}


def _get_runner(ns=NS, ncores=NCORES, z_f32r=False, reps=1):
    """Cached jitted executable (same lowering run_bass_kernel_spmd uses
    under axon, but jitted once so repeat calls don't re-trace)."""
    key = (ns, ncores, z_f32r, reps)
    if key in _RUNNERS:
        return _RUNNERS[key]
    import jax
    from jax.experimental.shard_map import shard_map
    from jax.sharding import Mesh, PartitionSpec
    from concourse import bass2jax, mybir

    nc = _get_nc(ns, z_f32r, 99, reps)
    bass2jax.install_neuronx_cc_hook()

    pname = nc.partition_id_tensor.name if nc.partition_id_tensor else None
    in_names, out_names, out_avals = [], [], []
    for alloc in nc.m.functions[0].allocations:
        if not isinstance(alloc, mybir.MemoryLocationSet):
            continue
        name = alloc.memorylocations[0].name
        if alloc.kind == "ExternalInput":
            if name != pname:
                in_names.append(name)
        elif alloc.kind == "ExternalOutput":
            out_names.append(name)
            out_avals.append(jax.core.ShapedArray(
                tuple(alloc.tensor_shape), mybir.dt.np(alloc.dtype)))
    n_params = len(in_names)
    all_names = in_names + out_names + ([pname] if pname else [])

    def _body(*args):
        operands = list(args)
        if pname is not None:
            operands.append(bass2jax.partition_id_tensor())
        outs = bass2jax._bass_exec_p.bind(
            *operands, out_avals=tuple(out_avals), in_names=tuple(all_names),
            out_names=tuple(out_names), lowering_input_output_aliases=(),
            sim_require_finite=True, sim_require_nnan=True, nc=nc)
        return tuple(outs)

    devices = jax.devices()[:ncores]
    mesh = Mesh(np.asarray(devices), ("core",))
    nio = n_params + len(out_names)
    jitted = jax.jit(
        shard_map(_body, mesh=mesh, in_specs=(PartitionSpec("core"),) * nio,
                  out_specs=(PartitionSpec("core"),) * len(out_names),
                  check_rep=False),
        donate_argnums=tuple(range(n_params, nio)), keep_unused=True)
    _RUNNERS[key] = (jitted, in_names, out_names, out_avals)
    return _RUNNERS[key]


def _prep_inputs(x, A_prev, gamma):
    x = np.ascontiguousarray(x, dtype=np.float32)
    A_prev = np.ascontiguousarray(A_prev, dtype=np.float32)
    ident = np.eye(C, dtype=np.float32)
    gvec = np.full((C, 1), np.float32(gamma), dtype=np.float32)
    return {"x": x, "a_prev": A_prev,
            "ident": np.concatenate([ident] * NCORES, 0),
            "gvec": np.concatenate([gvec] * NCORES, 0)}


def _run_fast(inp, ns=NS, ncores=NCORES, z_f32r=True, reps=1):
    """inp: dict from _prep_inputs (or jax device arrays). Returns jax arrays."""
    import jax.numpy as jnp
    jitted, in_names, out_names, out_avals = _get_runner(ns, ncores,
                                                         z_f32r, reps)
    args = [inp[n] for n in in_names]
    zeros = [jnp.zeros((ncores * a.shape[0],) + tuple(a.shape[1:]), a.dtype)
             for a in out_avals]
    outs = jitted(*args, *zeros)
    return {n: o for n, o in zip(out_names, outs)}


def kernel(x, A_prev, gamma):
    outs = _run_fast(_prep_inputs(x, A_prev, gamma), z_f32r=True)
    return (np.asarray(outs["x_out"]), np.asarray(outs["a_out"]))


# revision 48
# speedup vs baseline: 1.0008x; 1.0008x over previous
"""Trainium2 Bass kernel for DynGraphBlock (gnn_message_passing).

Per sample n (x: [C=128, T=2048], A_prev: [C, C], gamma scalar):
  1. corr affinity  A0 = relu(corr(x))           (Pearson corr over T, ddof=1)
  2. top-8 sparsify per row
  3. normalize: sym + self-loop + row-normalize
  4. EMA with A_prev -> A
  5. x_out = x + gamma * (A @ x)
Returns (x_out, A).

Mapping:
  - Pure data parallel: 256 samples -> 8 NeuronCores x 32 samples.
  - Gram G = x @ x.T via PE with x transposed on PE (16 [128,128] transposes),
    augmented with a ones column so the same matmuls also produce row sums S.
    corr = (G - u u^T) * w_c * w_k / T with u = S/sqrt(T), w = 1/(std+eps);
    u/w rows come from a PE transpose + gpsimd partition_broadcast, and the
    rank-1 mean correction and scaling run on DVE.
  - top-8 threshold via the DVE Max (top-8 per partition) instruction: keep
    entries >= 8th largest (ties only occur among zero/negative entries,
    where the masked product is 0 either way, matching jax top_k).
  - x_out = x + gamma*A@x is a single PE matmul with W = gamma*A^T + I; it
    runs in fp32r (tf32) on a rounded copy of x (A/corr keep full fp32).
  - Tile priority nudges: per-sample stats promoted, post-Gram tail demoted,
    so PE stays busy with the next sample's transposes/Gram during the
    current sample's DVE chain.
"""

import numpy as np
import sys

if "/opt/trn_rl_repo" not in sys.path:
    sys.path.insert(0, "/opt/trn_rl_repo")

N, C, T = 256, 128, 2048
NCORES = 8
NS = N // NCORES            # samples per core
TOPK = 8
EMA_ALPHA = 0.8
EPS = 1e-6
TB = T // C                 # 16 t-blocks of 128
ZCHUNK = 512                # moving free dim per x_out matmul
SQT = float(np.sqrt(np.float64(T)))

_CACHE = {}


def _build(ns, z_f32r=False, stage=99, reps=1):
    import concourse.bacc as bacc
    import concourse.mybir as mybir
    import concourse.tile as tile

    f32 = mybir.dt.float32
    f32r = mybir.dt.float32r
    op = mybir.AluOpType

    nc = bacc.Bacc("TRN2", target_bir_lowering=False, debug=False,
                   enable_asserts=False, num_devices=NCORES)
    x_d = nc.dram_tensor("x", [ns, C, T], f32, kind="ExternalInput")
    ap_d = nc.dram_tensor("a_prev", [ns, C, C], f32, kind="ExternalInput")
    id_d = nc.dram_tensor("ident", [C, C], f32, kind="ExternalInput")
    gv_d = nc.dram_tensor("gvec", [C, 1], f32, kind="ExternalInput")
    xo_d = nc.dram_tensor("x_out", [ns, C, T], f32, kind="ExternalOutput")
    ao_d = nc.dram_tensor("a_out", [ns, C, C], f32, kind="ExternalOutput")

    with tile.TileContext(nc) as tc:
        with (
            tc.tile_pool(name="const", bufs=1) as constp,
            tc.tile_pool(name="xin", bufs=4) as xinp,
            tc.tile_pool(name="xt", bufs=4) as xtp,
            tc.tile_pool(name="apin", bufs=3) as apinp,
            tc.tile_pool(name="xout", bufs=3) as xoutp,
            tc.tile_pool(name="wrk", bufs=5) as wrk,
            tc.tile_pool(name="tp_ps", bufs=3, space="PSUM") as tpps,
            tc.tile_pool(name="g_ps", bufs=2, space="PSUM") as gps,
            tc.tile_pool(name="st_ps", bufs=1, space="PSUM") as stps,
            tc.tile_pool(name="z_ps", bufs=2, space="PSUM") as zps,
        ):
            ident = constp.tile([C, C], f32, tag="ident")
            nc.sync.dma_start(ident[:], id_d.ap())
            gvec = constp.tile([C, 1], f32, tag="gvec")
            nc.sync.dma_start(gvec[:], gv_d.ap())

            for _it in range(reps * ns):
                s = _it % ns
                # ---- loads ----
                x_sb = xinp.tile([C, T], f32, tag="x")
                nc.sync.dma_start(x_sb[:], x_d.ap()[s])
                if z_f32r:
                    # tf32-rounded copy of x for the fast x_out matmul; the
                    # Gram/corr path keeps full fp32 x. gpsimd is idle.
                    x_r = xinp.tile([C, T], f32r, tag="xr")
                    nc.gpsimd.tensor_copy(x_r[:], x_sb[:])
                ap_sb = apinp.tile([C, C], f32, tag="ap")
                nc.sync.dma_start(ap_sb[:], ap_d.ap()[s])
                ap8 = wrk.tile([C, C], f32, tag="ap8")
                nc.vector.tensor_scalar(ap8[:], ap_sb[:], EMA_ALPHA, None,
                                        op.mult)

                # ---- transpose x into xT blocks (+ ones columns) ----
                # xT layout: 16 blocks of 129 cols: [128 cols x^T block | 1.0]
                xT = xtp.tile([C, TB * (C + 1)], f32, tag="xT")
                xT3 = xT.rearrange("p (b c) -> p b c", c=C + 1)
                nc.gpsimd.memset(xT3[:, :, C:C + 1], 1.0)
                for g in range(TB // 4):
                    tp = tpps.tile([C, 4 * C], f32, tag="tp")
                    for j in range(4):
                        blk = 4 * g + j
                        nc.tensor.transpose(
                            tp[:, j * C:(j + 1) * C],
                            x_sb[:, blk * C:(blk + 1) * C], ident[:])
                    src = tp.rearrange("p (b c) -> p b c", c=C)
                    dst = xT3[:, 4 * g:4 * (g + 1), 0:C]
                    if g % 2 == 0:
                        nc.vector.tensor_copy(dst, src)
                    else:
                        nc.scalar.copy(dst, src)

                # ---- Gram (+ row sums in col 128) ----
                G = gps.tile([C, C + 1], f32, tag="g")
                for k in range(TB):
                    nc.tensor.matmul(
                        G[:], xT[:, k * (C + 1):k * (C + 1) + C],
                        xT[:, k * (C + 1):k * (C + 1) + C + 1],
                        start=(k == 0), stop=(k == TB - 1),
                        skip_group_check=True)

                if stage <= 1:
                    dmp = wrk.tile([C, C], f32, tag="dmp")
                    nc.vector.tensor_copy(dmp[:], G[:, 0:C])
                    nc.sync.dma_start(ao_d.ap()[s], dmp[:])
                    xo_sb = xoutp.tile([C, T], f32, tag="xo")
                    nc.vector.tensor_copy(xo_sb[:], x_sb[:])
                    nc.sync.dma_start(xo_d.ap()[s], xo_sb[:])
                    continue

                # ---- stats: u = S/sqrt(T), w = 1/(std+eps) ----
                _sctx = tc.high_priority(offset=35)
                _sctx.__enter__()
                scr = wrk.tile([C, C], f32, tag="scr")
                Q = wrk.tile([C, 1], f32, tag="q")
                nc.vector.tensor_tensor(scr[:], G[:, 0:C], ident[:], op.mult)
                nc.vector.reduce_sum(Q[:], scr[:], axis=mybir.AxisListType.X)
                uw = wrk.tile([C, 3], f32, tag="uw")
                # col1 = u = S/sqrt(T); col0 = -u; col2 = w
                nc.vector.tensor_scalar(uw[:, 1:2], G[:, C:C + 1], 1.0 / SQT,
                                        None, op.mult)
                sq = wrk.tile([C, 1], f32, tag="sq")
                nc.vector.tensor_tensor(sq[:], uw[:, 1:2], uw[:, 1:2], op.mult)
                varnum = wrk.tile([C, 1], f32, tag="varnum")
                nc.vector.tensor_tensor(varnum[:], Q[:], sq[:], op.subtract)
                stdv = wrk.tile([C, 1], f32, tag="stdv")
                nc.scalar.activation(stdv[:], varnum[:],
                                     mybir.ActivationFunctionType.Sqrt,
                                     bias=0.0, scale=1.0 / (T - 1))
                stde = wrk.tile([C, 1], f32, tag="stde")
                nc.vector.tensor_scalar(stde[:], stdv[:], EPS, None, op.add)
                nc.vector.reciprocal(uw[:, 2:3], stde[:])
                _sctx.__exit__(None, None, None)
                _pctx = tc.high_priority(offset=-30)
                _pctx.__enter__()

                # ---- rows: transpose u, w columns to [1, 128] rows ----
                R = stps.tile([1, 2 * C], f32, tag="st")
                for jj, j in enumerate((1, 2)):
                    nc.tensor.transpose(R[0:1, jj * C:(jj + 1) * C],
                                        uw[:, j:j + 1], ident[:])
                uwrow = wrk.tile([1, 2 * C], f32, tag="uwrow")
                nc.scalar.copy(uwrow[:], R[:])

                if stage <= 2:
                    dmp = wrk.tile([C, C], f32, tag="dmp")
                    nc.vector.tensor_copy(dmp[:], G[:, 0:C])
                    nc.sync.dma_start(ao_d.ap()[s], dmp[:])
                    xo_sb = xoutp.tile([C, T], f32, tag="xo")
                    nc.vector.tensor_copy(xo_sb[:], x_sb[:])
                    nc.sync.dma_start(xo_d.ap()[s], xo_sb[:])
                    continue

                # Urep[c, k] = u_k ; Wrep[c, k] = w_k
                urep = wrk.tile([C, C], f32, tag="urep")
                nc.gpsimd.partition_broadcast(urep[:], uwrow[0:1, 0:C])
                wrep = wrk.tile([C, C], f32, tag="wrep")
                nc.gpsimd.partition_broadcast(wrep[:], uwrow[0:1, C:2 * C])

                # ---- Gc = G - u u^T ; t2 = Gc * w_k ; top-8 threshold ----
                uu = wrk.tile([C, C], f32, tag="uu")
                nc.vector.tensor_scalar(uu[:], urep[:], uw[:, 1:2], None,
                                        op.mult)
                gc = wrk.tile([C, C], f32, tag="gc")
                nc.vector.tensor_tensor(gc[:], G[:, 0:C], uu[:], op.subtract)
                t2 = wrk.tile([C, C], f32, tag="t2")
                nc.vector.tensor_tensor(t2[:], gc[:], wrep[:], op.mult)
                top8 = wrk.tile([C, 8], f32, tag="top8")
                nc.vector.max(top8[:], t2[:])
                mask = wrk.tile([C, C], f32, tag="mask")
                nc.vector.tensor_scalar(mask[:], t2[:], top8[:, 7:8], None,
                                        op.is_ge)
                m1 = wrk.tile([C, C], f32, tag="m1")
                nc.vector.tensor_tensor(m1[:], t2[:], mask[:], op.mult)
                # Asp = relu(m1 * (w_c / T))
                w0T = wrk.tile([C, 1], f32, tag="w0T")
                nc.vector.tensor_scalar(w0T[:], uw[:, 2:3], 1.0 / T, None,
                                        op.mult)
                Asp = wrk.tile([C, C], f32, tag="asp")
                nc.vector.tensor_scalar(Asp[:], m1[:], w0T[:], 0.0,
                                        op.mult, op.max)

                if stage <= 3:
                    nc.sync.dma_start(ao_d.ap()[s], Asp[:])
                    xo_sb = xoutp.tile([C, T], f32, tag="xo")
                    nc.vector.tensor_copy(xo_sb[:], x_sb[:])
                    nc.sync.dma_start(xo_d.ap()[s], xo_sb[:])
                    continue

                # ---- symmetrize + degree ----
                AspT = stps.tile([C, C], f32, tag="st")
                nc.tensor.transpose(AspT[:], Asp[:], ident[:])
                S2 = wrk.tile([C, C], f32, tag="s2")
                nc.vector.tensor_tensor(S2[:], Asp[:], AspT[:], op.add)
                M05 = wrk.tile([C, C], f32, tag="m05")
                nc.vector.tensor_scalar(M05[:], S2[:], 0.5, None, op.mult)
                deg0 = wrk.tile([C, 1], f32, tag="deg0")
                nc.vector.reduce_sum(deg0[:], S2[:], axis=mybir.AxisListType.X)
                # deg = 0.5 * rowsum(S2) + 1 (self-loop) + EPS
                degi = wrk.tile([C, 1], f32, tag="degi")
                nc.vector.tensor_scalar(degi[:], deg0[:], 0.5, 1.0 + EPS,
                                        op.mult, op.add)
                dinv = wrk.tile([C, 1], f32, tag="dinv")
                nc.vector.reciprocal(dinv[:], degi[:])
                d02 = wrk.tile([C, 1], f32, tag="d02")
                nc.vector.tensor_scalar(d02[:], dinv[:], 1.0 - EMA_ALPHA, None,
                                        op.mult)

                # ---- EMA: Afin = alpha*A_prev + (M05 + I) * d02_c ----
                f1 = wrk.tile([C, C], f32, tag="f1")
                nc.vector.tensor_scalar(f1[:], M05[:], d02[:], None, op.mult)
                f2 = wrk.tile([C, C], f32, tag="f2")
                nc.vector.tensor_scalar(f2[:], ident[:], d02[:], None, op.mult)
                f3 = wrk.tile([C, C], f32, tag="f3")
                nc.vector.tensor_tensor(f3[:], f1[:], f2[:], op.add)
                Afin = wrk.tile([C, C], f32, tag="afin")
                nc.vector.tensor_tensor(Afin[:], f3[:], ap8[:], op.add)
                nc.sync.dma_start(ao_d.ap()[s], Afin[:])

                if stage <= 4:
                    xo_sb = xoutp.tile([C, T], f32, tag="xo")
                    nc.vector.tensor_copy(xo_sb[:], x_sb[:])
                    nc.sync.dma_start(xo_d.ap()[s], xo_sb[:])
                    continue

                # ---- W = gamma * Afin^T + I ; x_out = W^T @ x ----
                AfinT = stps.tile([C, C], f32, tag="st")
                nc.tensor.transpose(AfinT[:], Afin[:], ident[:])
                Wt = wrk.tile([C, C], f32, tag="wt")
                nc.vector.tensor_scalar(Wt[:], AfinT[:], gvec[:], None,
                                        op.mult)
                W = wrk.tile([C, C], f32r if z_f32r else f32, tag="w")
                nc.vector.tensor_tensor(W[:], Wt[:], ident[:], op.add)

                xo_sb = xoutp.tile([C, T], f32, tag="xo")
                xz = x_r if z_f32r else x_sb
                for j in range(T // ZCHUNK):
                    zp = zps.tile([C, ZCHUNK], f32, tag="z")
                    nc.tensor.matmul(
                        zp[:], W[:],
                        xz[:, j * ZCHUNK:(j + 1) * ZCHUNK],
                        start=True, stop=True)
                    nc.scalar.copy(xo_sb[:, j * ZCHUNK:(j + 1) * ZCHUNK],
                                   zp[:])
                nc.sync.dma_start(xo_d.ap()[s], xo_sb[:])
                _pctx.__exit__(None, None, None)

    nc.compile()
    return nc


def _get_nc(ns=NS, z_f32r=False, stage=99, reps=1):
    key = (ns, z_f32r, stage, reps)
    if key not in _CACHE:
        _CACHE[key] = _build(ns, z_f32r, stage, reps)
    return _CACHE[key]


def _run(x, A_prev, gamma, ns, ncores, z_f32r=False, trace=False, stage=99):
    from concourse.bass_utils import run_bass_kernel_spmd

    nc = _get_nc(ns, z_f32r, stage)
    x = np.ascontiguousarray(x, dtype=np.float32)
    A_prev = np.ascontiguousarray(A_prev, dtype=np.float32)
    ident = np.eye(C, dtype=np.float32)
    gvec = np.full((C, 1), np.float32(gamma), dtype=np.float32)
    in_maps = []
    for c in range(ncores):
        sl = slice(c * ns, (c + 1) * ns)
        in_maps.append({"x": x[sl], "a_prev": A_prev[sl],
                        "ident": ident, "gvec": gvec})
    res = run_bass_kernel_spmd(nc, in_maps, core_ids=list(range(ncores)),
                               trace=trace)
    x_out = np.concatenate([res.results[c]["x_out"] for c in range(ncores)], 0)
    a_out = np.concatenate([res.results[c]["a_out"] for c in range(ncores)], 0)
    return (x_out, a_out), res


_RUNNERS = {}


def _get_runner(ns=NS, ncores=NCORES, z_f32r=False, reps=1):
    """Cached jitted executable (same lowering run_bass_kernel_spmd uses
    under axon, but jitted once so repeat calls don't re-trace)."""
    key = (ns, ncores, z_f32r, reps)
    if key in _RUNNERS:
        return _RUNNERS[key]
    import jax
    from jax.experimental.shard_map import shard_map
    from jax.sharding import Mesh, PartitionSpec
    from concourse import bass2jax, mybir

    nc = _get_nc(ns, z_f32r, 99, reps)
    bass2jax.install_neuronx_cc_hook()

    pname = nc.partition_id_tensor.name if nc.partition_id_tensor else None
    in_names, out_names, out_avals = [], [], []
    for alloc in nc.m.functions[0].allocations:
        if not isinstance(alloc, mybir.MemoryLocationSet):
            continue
        name = alloc.memorylocations[0].name
        if alloc.kind == "ExternalInput":
            if name != pname:
                in_names.append(name)
        elif alloc.kind == "ExternalOutput":
            out_names.append(name)
            out_avals.append(jax.core.ShapedArray(
                tuple(alloc.tensor_shape), mybir.dt.np(alloc.dtype)))
    n_params = len(in_names)
    all_names = in_names + out_names + ([pname] if pname else [])

    def _body(*args):
        operands = list(args)
        if pname is not None:
            operands.append(bass2jax.partition_id_tensor())
        outs = bass2jax._bass_exec_p.bind(
            *operands, out_avals=tuple(out_avals), in_names=tuple(all_names),
            out_names=tuple(out_names), lowering_input_output_aliases=(),
            sim_require_finite=True, sim_require_nnan=True, nc=nc)
        return tuple(outs)

    devices = jax.devices()[:ncores]
    mesh = Mesh(np.asarray(devices), ("core",))
    nio = n_params + len(out_names)
    jitted = jax.jit(
        shard_map(_body, mesh=mesh, in_specs=(PartitionSpec("core"),) * nio,
                  out_specs=(PartitionSpec("core"),) * len(out_names),
                  check_rep=False),
        donate_argnums=tuple(range(n_params, nio)), keep_unused=True)
    _RUNNERS[key] = (jitted, in_names, out_names, out_avals)
    return _RUNNERS[key]


def _prep_inputs(x, A_prev, gamma):
    x = np.ascontiguousarray(x, dtype=np.float32)
    A_prev = np.ascontiguousarray(A_prev, dtype=np.float32)
    ident = np.eye(C, dtype=np.float32)
    gvec = np.full((C, 1), np.float32(gamma), dtype=np.float32)
    return {"x": x, "a_prev": A_prev,
            "ident": np.concatenate([ident] * NCORES, 0),
            "gvec": np.concatenate([gvec] * NCORES, 0)}


def _run_fast(inp, ns=NS, ncores=NCORES, z_f32r=True, reps=1):
    """inp: dict from _prep_inputs (or jax device arrays). Returns jax arrays."""
    import jax.numpy as jnp
    jitted, in_names, out_names, out_avals = _get_runner(ns, ncores,
                                                         z_f32r, reps)
    args = [inp[n] for n in in_names]
    zeros = [jnp.zeros((ncores * a.shape[0],) + tuple(a.shape[1:]), a.dtype)
             for a in out_avals]
    outs = jitted(*args, *zeros)
    return {n: o for n, o in zip(out_names, outs)}


def kernel(x, A_prev, gamma):
    outs = _run_fast(_prep_inputs(x, A_prev, gamma), z_f32r=True)
    return (np.asarray(outs["x_out"]), np.asarray(outs["a_out"]))
